# revision 27
# baseline (speedup 1.0000x reference)
"""DiscreteFlow (MADE masked-MLP log-likelihood) on 8 Trainium2 NeuronCores.

Math (per batch row b):
    oh   = onehot(x)                  [T=1024]  (16 blocks of 64)
    h1   = relu(oh[:960] @ (W1*M1) + b1)
    h2   = relu(h1 @ (W2*M2) + b2)
    lg   = h2 @ (W3*M3) + b3          [1024]
    out  = sum_d lg[64d + x_d]  -  sum_d log(sum_k exp(lg[64d + k]))

Kernel layout: "transposed" dataflow — features live on SBUF partitions,
batch on the free axis.  All matmuls then take the stored (pre-masked,
host-side) weights directly as lhsT, biases are per-partition ACT scalars,
and no on-chip transposes are needed.  The per-block exp-sum and the final
per-batch reductions are partition reductions, done as tiny PE matmuls
(block-indicator / ones / -ones stationary operands) accumulating into one
PSUM bank per 512-batch chunk.

The three dense matmul chains run in fp8(e4m3) DoubleRow (2 contraction
rows/cycle, fp32 PSUM accumulate).  To stay out of e4m3's subnormal range,
weights are pre-scaled x32 on host and activations x8 on-chip; the scales
are folded into the (free) ACT scale/bias of each layer epilogue, so the
logits seen by exp/gather are exact up to fp8/bf16 rounding.  The small
reduction matmuls stay bf16, and ln(norms/64) keeps values tiny so bf16 is
safe there too; the constant 16*ln(64) is re-added in the final bias.

Sharding: pure data parallel, 4096 batch rows per core, weights replicated.
"""

from contextlib import ExitStack

import ml_dtypes
import numpy as np

import concourse.bass as bass
import concourse.tile as tile
from concourse import bacc, mybir
from concourse.bass_utils import run_bass_kernel_spmd

F32 = mybir.dt.float32
F16 = mybir.dt.float16
BF16 = mybir.dt.bfloat16
FP8 = mybir.dt.float8e4
BF16_NP = ml_dtypes.bfloat16
FP8_NP = ml_dtypes.float8_e4m3

D, K, T, H = 16, 64, 1024, 1024
B = 32768
NCORES = 8
BC = B // NCORES  # 4096 batch rows per core
P = 128
NKT = T // P  # 8 feature tiles of 128 (same for H)
NKP = NKT // 2  # 4 DoubleRow pair-tiles of 256
WS = 32.0  # host weight prescale (keeps fp8 weights normal-range)
HS = 8.0  # on-chip activation prescale
DR = mybir.MatmulPerfMode.DoubleRow


def _emit(tc, t, BC_, NSC, NCH):
    """Emit the per-core program.  t: dict name -> dram handle."""
    nc = tc.nc
    ctx = ExitStack()
    n_sc = BC_ // NSC
    n_ch = NSC // NCH

    consts = ctx.enter_context(tc.tile_pool(name="consts", bufs=1))
    wpool = ctx.enter_context(tc.tile_pool(name="w", bufs=1))
    ohp = ctx.enter_context(tc.tile_pool(name="ohp", bufs=2))
    h1p = ctx.enter_context(tc.tile_pool(name="h1p", bufs=1))
    h2p = ctx.enter_context(tc.tile_pool(name="h2p", bufs=1))
    exps = ctx.enter_context(tc.tile_pool(name="exps", bufs=5))
    prods = ctx.enter_context(tc.tile_pool(name="prods", bufs=5))
    lns = ctx.enter_context(tc.tile_pool(name="lns", bufs=2))
    osb = ctx.enter_context(tc.tile_pool(name="osb", bufs=2))
    psmm = ctx.enter_context(tc.tile_pool(name="psmm", bufs=4, space="PSUM"))
    psn = ctx.enter_context(tc.tile_pool(name="psn", bufs=2, space="PSUM"))
    pso = ctx.enter_context(tc.tile_pool(name="pso", bufs=2, space="PSUM"))

    # ---- constants / weights into SBUF (once) ----
    # blk16[:, 16m + r] = (r == 2m + p//64): per-m indicator whose matmul
    # against ex[m] lands that m's two block-norm rows in a shared [16, NCH]
    # PSUM tile (accumulation packs partitions engines can't).
    blk16 = consts.tile([P, NKT * 16], BF16, name="blk16")
    nc.sync.dma_start(out=blk16[:], in_=t["blk16"][:])
    b1s = consts.tile([P, NKT], F32, name="b1s")  # pre-scaled x HS on host
    nc.sync.dma_start(out=b1s[:], in_=t["b1r"][:])
    b2s = consts.tile([P, NKT], F32, name="b2s")  # pre-scaled x HS on host
    nc.sync.dma_start(out=b2s[:], in_=t["b2r"][:])
    b3f = consts.tile([P, NKT], F32, name="b3f")
    nc.sync.dma_start(out=b3f[:], in_=t["b3f"][:])
    ones128 = consts.tile([P, 1], BF16, name="ones128")
    nc.vector.memset(ones128[:], 1.0)
    negones16 = consts.tile([16, 1], BF16, name="negones16")
    nc.vector.memset(negones16[:], -1.0)
    negk = consts.tile([1, 1], F32, name="negk")
    nc.vector.memset(negk[:], float(-D * np.log(K)))

    # weights: [NKP, 128, 2, H] fp8, DoubleRow plane j = contraction rows
    # 128*(2k'+j)+p (pre-masked, pre-scaled, pre-packed on host)
    wt = {}
    for wi, wname in ((1, "w1"), (2, "w2"), (3, "w3")):
        for kp in range(NKP):
            w = wpool.tile([P, 2, H], FP8, name=f"w{wi}_{kp}", tag=f"w{wi}_{kp}")
            nc.gpsimd.dma_start(out=w[:], in_=t[wname][kp * P : (kp + 1) * P, :, :])
            wt[wi, kp] = w

    # finish(c): the deferred per-chunk reduction — one Ln over the gathered
    # [2, 8*NCH] norm strip, 8 accumulate matmuls, final bias, DMA out.
    # Deferred behind the NEXT chunk's dense matmuls so the PE never stalls
    # on the ACT Ln; survives superchunk boundaries.
    pending_finish = [None]

    def emit_finish():
        if pending_finish[0] is None:
            return
        s_, c_, ops_, pn16_ = pending_finish[0]
        pending_finish[0] = None
        lnt = lns.tile([16, NCH], BF16, name=f"ln_{s_}_{c_}", tag="ln")
        # ln(norms/64): tiny values, bf16-safe; 16*ln(64) folded into the
        # final bias below.
        nc.scalar.activation(
            lnt[:], pn16_[:], mybir.ActivationFunctionType.Ln, scale=1.0 / K
        )
        nc.tensor.matmul(ops_[:], negones16[:], lnt[:], start=False, stop=True)
        ob = osb.tile([1, NCH], F32, name=f"ob_{s_}_{c_}", tag="ob")
        nc.vector.tensor_scalar(ob[:], ops_[:], negk[:], None, mybir.AluOpType.add)
        g = s_ * n_ch + c_
        nc.sync.dma_start(out=t["out"][g : g + 1, :], in_=ob[:])

    def mlp_layer(in_tiles, wi, bias_sb, outpool, tag, act_scale):
        """Dense fp8 DoubleRow layer: out[m] = relu(psum*act_scale + b[m]).

        in_tiles: NKP tiles [128, 2, NSC]; returns same-shaped output tiles.
        """
        outs = [
            outpool.tile([P, 2, NSC], FP8, name=f"{tag}{i}", tag=f"{tag}{i}")
            for i in range(NKP)
        ]
        for m in range(NKT):
            pss = []
            for c in range(n_ch):
                ps = psmm.tile([P, NCH], F32, name=f"ps_{tag}{m}_{c}", tag="ps")
                pss.append(ps)
            for kp in range(NKP):
                lhsT = wt[wi, kp][:, :, m * P : (m + 1) * P]
                for c in range(n_ch):
                    nc.tensor.matmul(
                        pss[c][:],
                        lhsT,
                        in_tiles[kp][:, :, c * NCH : (c + 1) * NCH],
                        start=(kp == 0),
                        stop=(kp == NKP - 1),
                        perf_mode=DR,
                    )
            for c in range(n_ch):
                nc.scalar.activation(
                    outs[m // 2][:, m % 2, c * NCH : (c + 1) * NCH],
                    pss[c][:],
                    mybir.ActivationFunctionType.Relu,
                    bias=bias_sb[:, m : m + 1],
                    scale=act_scale,
                )
        return outs

    for s in range(n_sc):
        # ---- phase A: one-hot arrives from host in DoubleRow fp8 layout ----
        # (ohp bufs=2 => superchunk s+1 prefetches during s on the idle ring)
        oh = [
            ohp.tile([P, 2, NSC], FP8, name=f"oh_{s}_{kp}", tag=f"oh{kp}")
            for kp in range(NKP)
        ]
        rings = [nc.sync, nc.scalar]
        for kp in range(NKP):
            r0 = (s * NKP + kp) * P
            for c0 in range(n_ch):
                cs0 = slice(c0 * NCH, (c0 + 1) * NCH)
                rings[(kp * n_ch + c0) % 2].dma_start(
                    out=oh[kp][:, :, cs0], in_=t["ohdr"][r0 : r0 + P, :, cs0]
                )

        # ---- phases B, C: the two hidden layers ----
        # psum1 = oh @ (WS*W1)            -> h1 = HS*relu(pre1+b1): scale HS/WS
        # psum2 = (HS*h1) @ (WS*W2)       -> h2 = HS*relu(pre2+b2): scale 1/WS
        h1 = mlp_layer(oh, 1, b1s, h1p, "h1", HS / WS)
        h2 = mlp_layer(h1, 2, b2s, h2p, "h2", 1.0 / WS)

        # ---- phase D: logits, exp, block-norms, gather, final reduce ----
        # psum3 = (HS*h2) @ (WS*W3) = HS*WS * logits
        # Software-pipelined so the PE never waits on ACT/DVE round trips:
        #  - stage(m): dense logits matmuls + (ACT exp, DVE scale+b3, DVE *oh)
        #  - tail(m):  dependent tiny PE matmuls, emitted one m behind
        lgs = 1.0 / (HS * WS)
        for c in range(n_ch):
            cs = slice(c * NCH, (c + 1) * NCH)
            ops = pso.tile([1, NCH], F32, name=f"ops_{s}_{c}", tag="ops")
            pn16 = psn.tile([16, NCH], F32, name=f"pn16_{s}_{c}", tag="pn16")
            exl, prl = {}, {}

            def stage(m):
                ps = psmm.tile([P, NCH], F32, name=f"lg_{s}_{c}_{m}", tag="ps")
                for kp in range(NKP):
                    nc.tensor.matmul(
                        ps[:],
                        wt[3, kp][:, :, m * P : (m + 1) * P],
                        h2[kp][:, :, cs],
                        start=(kp == 0),
                        stop=(kp == NKP - 1),
                        perf_mode=DR,
                    )
                # psum `ps` = HS*WS*logits (no b3); b3 enters via the Exp
                # bias (per-partition f32) and the DVE scale+add below.
                ex = exps.tile([P, NCH], BF16, name=f"ex_{s}_{c}_{m}", tag="ex")
                nc.scalar.activation(
                    ex[:],
                    ps[:],
                    mybir.ActivationFunctionType.Exp,
                    bias=b3f[:, m : m + 1],
                    scale=lgs,
                )
                tmp = prods.tile([P, NCH], BF16, name=f"tmp_{s}_{c}_{m}", tag="tmp")
                nc.vector.tensor_scalar(
                    tmp[:],
                    ps[:],
                    lgs,
                    b3f[:, m : m + 1],
                    mybir.AluOpType.mult,
                    mybir.AluOpType.add,
                )
                pr = prods.tile([P, NCH], BF16, name=f"pr_{s}_{c}_{m}", tag="pr")
                nc.vector.tensor_mul(pr[:], tmp[:], oh[m // 2][:, m % 2, cs])
                exl[m], prl[m] = ex, pr

            def tail(m):
                nc.tensor.matmul(
                    pn16[:],
                    blk16[:, m * 16 : (m + 1) * 16],
                    exl[m][:],
                    start=(m == 0),
                    stop=(m == NKT - 1),
                )
                nc.tensor.matmul(
                    ops[:], ones128[:], prl[m][:], start=(m == 0), stop=False
                )

            stage(0)
            stage(1)
            stage(2)
            stage(3)
            tail(0)
            emit_finish()  # previous chunk's reduction, behind 4 fresh kloops
            for m in range(4, NKT):
                stage(m)
                tail(m - 3)
            tail(NKT - 3)
            tail(NKT - 2)
            tail(NKT - 1)
            pending_finish[0] = (s, c, ops, pn16)
    emit_finish()

    ctx.close()


def build_nc(BC_=BC, NSC=2048, NCH=512):
    nc = bacc.Bacc("TRN2", target_bir_lowering=False, debug=False)
    t = {
        "ohdr": nc.dram_tensor("ohdr", [(BC_ // NSC) * (T // 2), 2, NSC], FP8, kind="ExternalInput"),
        "w1": nc.dram_tensor("w1", [T // 2, 2, H], FP8, kind="ExternalInput"),
        "w2": nc.dram_tensor("w2", [H // 2, 2, H], FP8, kind="ExternalInput"),
        "w3": nc.dram_tensor("w3", [H // 2, 2, T], FP8, kind="ExternalInput"),
        "b1r": nc.dram_tensor("b1r", [P, NKT], F32, kind="ExternalInput"),
        "b2r": nc.dram_tensor("b2r", [P, NKT], F32, kind="ExternalInput"),
        "b3f": nc.dram_tensor("b3f", [P, NKT], F32, kind="ExternalInput"),
        "blk16": nc.dram_tensor("blk16", [P, NKT * 16], BF16, kind="ExternalInput"),
        "out": nc.dram_tensor("out", [BC_ // NCH, NCH], F32, kind="ExternalOutput"),
    }
    with tile.TileContext(nc) as tc:
        _emit(tc, t, BC_, NSC, NCH)
    nc.compile()
    return nc


def _made_masks_np():
    in_deg = np.repeat(np.arange(D - 1), K)
    hid_deg = np.arange(H) % (D - 1)
    out_deg = np.repeat(np.arange(D), K)
    M1 = (hid_deg[None, :] >= in_deg[:, None]).astype(np.float32)
    M2 = (hid_deg[None, :] >= hid_deg[:, None]).astype(np.float32)
    M3 = (out_deg[None, :] > hid_deg[:, None]).astype(np.float32)
    return M1, M2, M3


def _pack_dr(wm):
    """[1024, C] f32 -> [512, 2, C] fp8 DoubleRow plane layout:
    out[128*kp + p, j, c] = WS * wm[128*(2*kp + j) + p, c]."""
    C = wm.shape[1]
    return np.ascontiguousarray(
        (WS * wm).reshape(NKP, 2, P, C).transpose(0, 2, 1, 3).reshape(NKP * P, 2, C)
    ).astype(FP8_NP)


def host_inputs(x, W1, b1, W2, b2, W3, b3, BC_=BC, n_cores=NCORES, NSC=2048):
    """Build the per-core in_maps (host-side prep: mask weights, expand x)."""
    x = np.asarray(x)
    M1, M2, M3 = _made_masks_np()
    w1m = np.zeros((H, H), dtype=np.float32)
    w1m[: T - K] = np.asarray(W1, np.float32) * M1
    w2m = np.asarray(W2, np.float32) * M2
    w3m = np.asarray(W3, np.float32) * M3
    b1r = (HS * np.asarray(b1, np.float32)).reshape(NKT, P).T.copy()
    b2r = (HS * np.asarray(b2, np.float32)).reshape(NKT, P).T.copy()
    b3c = np.asarray(b3, np.float32).reshape(NKT, P).T.copy()
    iota = (np.arange(T) % K).astype(np.int32)
    pp = np.arange(P) // K  # 0 for partitions 0..63, 1 for 64..127
    blk16 = np.zeros((P, NKT * 16), np.float32)
    for m in range(NKT):
        blk16[np.arange(P), 16 * m + 2 * m + pp] = 1.0
    blk16 = blk16.astype(BF16_NP)

    in_maps = []
    for c in range(n_cores):
        xs = x[c * BC_ : (c + 1) * BC_]  # [BC, D]
        xrep = np.repeat(xs.T.astype(np.int32), K, axis=0)  # [T, BC]
        ohf = (xrep == iota[:, None]).astype(FP8_NP)  # exact 0/1 one-hot
        # per-superchunk contiguous DoubleRow blocks:
        # rows (s*NKP+kp)*P + p, plane j, col n  <-  ohf[128*(2kp+j)+p, s*NSC+n]
        n_sc = BC_ // NSC
        ohdr = np.ascontiguousarray(
            ohf.reshape(NKP, 2, P, n_sc, NSC)
            .transpose(3, 0, 2, 1, 4)
            .reshape(n_sc * NKP * P, 2, NSC)
        )
        in_maps.append(
            {
                "ohdr": ohdr,
                "w1": _pack_dr(w1m),
                "w2": _pack_dr(w2m),
                "w3": _pack_dr(w3m),
                "b1r": b1r,
                "b2r": b2r,
                "b3f": b3c,
                "blk16": blk16,
            }
        )
    return in_maps


_NC_CACHE = {}


def kernel(x, W1, b1, W2, b2, W3, b3, **run_kwargs):
    if "nc" not in _NC_CACHE:
        _NC_CACHE["nc"] = build_nc()
    nc = _NC_CACHE["nc"]
    in_maps = host_inputs(x, W1, b1, W2, b2, W3, b3)
    res = run_bass_kernel_spmd(nc, in_maps, core_ids=list(range(NCORES)), **run_kwargs)
    out = np.concatenate([r["out"].reshape(-1) for r in res.results])
    if run_kwargs:
        kernel.last_results = res
    return out


# revision 28
# speedup vs baseline: 1.0026x; 1.0026x over previous
"""DiscreteFlow (MADE masked-MLP log-likelihood) on 8 Trainium2 NeuronCores.

Math (per batch row b):
    oh   = onehot(x)                  [T=1024]  (16 blocks of 64)
    h1   = relu(oh[:960] @ (W1*M1) + b1)
    h2   = relu(h1 @ (W2*M2) + b2)
    lg   = h2 @ (W3*M3) + b3          [1024]
    out  = sum_d lg[64d + x_d]  -  sum_d log(sum_k exp(lg[64d + k]))

Kernel layout: "transposed" dataflow — features live on SBUF partitions,
batch on the free axis.  All matmuls then take the stored (pre-masked,
host-side) weights directly as lhsT, biases are per-partition ACT scalars,
and no on-chip transposes are needed.  The per-block exp-sum and the final
per-batch reductions are partition reductions, done as tiny PE matmuls
(block-indicator / ones / -ones stationary operands) accumulating into one
PSUM bank per 512-batch chunk.

The three dense matmul chains run in fp8(e4m3) DoubleRow (2 contraction
rows/cycle, fp32 PSUM accumulate).  To stay out of e4m3's subnormal range,
weights are pre-scaled x32 on host and activations x8 on-chip; the scales
are folded into the (free) ACT scale/bias of each layer epilogue, so the
logits seen by exp/gather are exact up to fp8/bf16 rounding.  The small
reduction matmuls stay bf16, and ln(norms/64) keeps values tiny so bf16 is
safe there too; the constant 16*ln(64) is re-added in the final bias.

Sharding: pure data parallel, 4096 batch rows per core, weights replicated.
"""

from contextlib import ExitStack

import ml_dtypes
import numpy as np

import concourse.bass as bass
import concourse.tile as tile
from concourse import bacc, mybir
from concourse.bass_utils import run_bass_kernel_spmd

F32 = mybir.dt.float32
F16 = mybir.dt.float16
BF16 = mybir.dt.bfloat16
FP8 = mybir.dt.float8e4
BF16_NP = ml_dtypes.bfloat16
FP8_NP = ml_dtypes.float8_e4m3

D, K, T, H = 16, 64, 1024, 1024
B = 32768
NCORES = 8
BC = B // NCORES  # 4096 batch rows per core
P = 128
NKT = T // P  # 8 feature tiles of 128 (same for H)
NKP = NKT // 2  # 4 DoubleRow pair-tiles of 256
WS = 32.0  # host weight prescale (keeps fp8 weights normal-range)
HS = 8.0  # on-chip activation prescale
DR = mybir.MatmulPerfMode.DoubleRow


def _emit(tc, t, BC_, NSC, NCH):
    """Emit the per-core program.  t: dict name -> dram handle."""
    nc = tc.nc
    ctx = ExitStack()
    n_sc = BC_ // NSC
    n_ch = NSC // NCH

    consts = ctx.enter_context(tc.tile_pool(name="consts", bufs=1))
    wpool = ctx.enter_context(tc.tile_pool(name="w", bufs=1))
    ohp = ctx.enter_context(tc.tile_pool(name="ohp", bufs=2))
    h1p = ctx.enter_context(tc.tile_pool(name="h1p", bufs=1))
    h2p = ctx.enter_context(tc.tile_pool(name="h2p", bufs=1))
    exps = ctx.enter_context(tc.tile_pool(name="exps", bufs=6))
    prods = ctx.enter_context(tc.tile_pool(name="prods", bufs=5))
    lns = ctx.enter_context(tc.tile_pool(name="lns", bufs=2))
    osb = ctx.enter_context(tc.tile_pool(name="osb", bufs=2))
    psmm = ctx.enter_context(tc.tile_pool(name="psmm", bufs=4, space="PSUM"))
    psn = ctx.enter_context(tc.tile_pool(name="psn", bufs=2, space="PSUM"))
    pso = ctx.enter_context(tc.tile_pool(name="pso", bufs=2, space="PSUM"))

    # ---- constants / weights into SBUF (once) ----
    # blk16[:, 16m + r] = (r == 2m + p//64): per-m indicator whose matmul
    # against ex[m] lands that m's two block-norm rows in a shared [16, NCH]
    # PSUM tile (accumulation packs partitions engines can't).
    blk16 = consts.tile([P, NKT * 16], BF16, name="blk16")
    nc.sync.dma_start(out=blk16[:], in_=t["blk16"][:])
    b1s = consts.tile([P, NKT], F32, name="b1s")  # pre-scaled x HS on host
    nc.sync.dma_start(out=b1s[:], in_=t["b1r"][:])
    b2s = consts.tile([P, NKT], F32, name="b2s")  # pre-scaled x HS on host
    nc.sync.dma_start(out=b2s[:], in_=t["b2r"][:])
    b3f = consts.tile([P, NKT], F32, name="b3f")
    nc.sync.dma_start(out=b3f[:], in_=t["b3f"][:])
    ones128 = consts.tile([P, 1], BF16, name="ones128")
    nc.vector.memset(ones128[:], 1.0)
    negones16 = consts.tile([16, 1], BF16, name="negones16")
    nc.vector.memset(negones16[:], -1.0)
    negk = consts.tile([1, 1], F32, name="negk")
    nc.vector.memset(negk[:], float(-D * np.log(K)))

    # weights: [NKP, 128, 2, H] fp8, DoubleRow plane j = contraction rows
    # 128*(2k'+j)+p (pre-masked, pre-scaled, pre-packed on host)
    wt = {}
    for wi, wname in ((1, "w1"), (2, "w2"), (3, "w3")):
        for kp in range(NKP):
            w = wpool.tile([P, 2, H], FP8, name=f"w{wi}_{kp}", tag=f"w{wi}_{kp}")
            nc.gpsimd.dma_start(out=w[:], in_=t[wname][kp * P : (kp + 1) * P, :, :])
            wt[wi, kp] = w

    # finish(c): the deferred per-chunk reduction — one Ln over the gathered
    # [2, 8*NCH] norm strip, 8 accumulate matmuls, final bias, DMA out.
    # Deferred behind the NEXT chunk's dense matmuls so the PE never stalls
    # on the ACT Ln; survives superchunk boundaries.
    pending_finish = [None]

    def emit_finish():
        if pending_finish[0] is None:
            return
        s_, c_, ops_, pn16_ = pending_finish[0]
        pending_finish[0] = None
        lnt = lns.tile([16, NCH], BF16, name=f"ln_{s_}_{c_}", tag="ln")
        # ln(norms/64): tiny values, bf16-safe; 16*ln(64) folded into the
        # final bias below.
        nc.scalar.activation(
            lnt[:], pn16_[:], mybir.ActivationFunctionType.Ln, scale=1.0 / K
        )
        nc.tensor.matmul(ops_[:], negones16[:], lnt[:], start=False, stop=True)
        ob = osb.tile([1, NCH], F32, name=f"ob_{s_}_{c_}", tag="ob")
        nc.vector.tensor_scalar(ob[:], ops_[:], negk[:], None, mybir.AluOpType.add)
        g = s_ * n_ch + c_
        nc.sync.dma_start(out=t["out"][g : g + 1, :], in_=ob[:])

    def mlp_layer(in_tiles, wi, bias_sb, outpool, tag, act_scale):
        """Dense fp8 DoubleRow layer: out[m] = relu(psum*act_scale + b[m]).

        in_tiles: NKP tiles [128, 2, NSC]; returns same-shaped output tiles.
        """
        outs = [
            outpool.tile([P, 2, NSC], FP8, name=f"{tag}{i}", tag=f"{tag}{i}")
            for i in range(NKP)
        ]
        for m in range(NKT):
            pss = []
            for c in range(n_ch):
                ps = psmm.tile([P, NCH], F32, name=f"ps_{tag}{m}_{c}", tag="ps")
                pss.append(ps)
            for kp in range(NKP):
                lhsT = wt[wi, kp][:, :, m * P : (m + 1) * P]
                for c in range(n_ch):
                    nc.tensor.matmul(
                        pss[c][:],
                        lhsT,
                        in_tiles[kp][:, :, c * NCH : (c + 1) * NCH],
                        start=(kp == 0),
                        stop=(kp == NKP - 1),
                        perf_mode=DR,
                    )
            for c in range(n_ch):
                nc.scalar.activation(
                    outs[m // 2][:, m % 2, c * NCH : (c + 1) * NCH],
                    pss[c][:],
                    mybir.ActivationFunctionType.Relu,
                    bias=bias_sb[:, m : m + 1],
                    scale=act_scale,
                )
        return outs

    for s in range(n_sc):
        # ---- phase A: one-hot arrives from host in DoubleRow fp8 layout ----
        # (ohp bufs=2 => superchunk s+1 prefetches during s on the idle ring)
        oh = [
            ohp.tile([P, 2, NSC], FP8, name=f"oh_{s}_{kp}", tag=f"oh{kp}")
            for kp in range(NKP)
        ]
        rings = [nc.sync, nc.scalar]
        for kp in range(NKP):
            r0 = (s * NKP + kp) * P
            for c0 in range(n_ch):
                cs0 = slice(c0 * NCH, (c0 + 1) * NCH)
                rings[(kp * n_ch + c0) % 2].dma_start(
                    out=oh[kp][:, :, cs0], in_=t["ohdr"][r0 : r0 + P, :, cs0]
                )

        # ---- phases B, C: the two hidden layers ----
        # psum1 = oh @ (WS*W1)            -> h1 = HS*relu(pre1+b1): scale HS/WS
        # psum2 = (HS*h1) @ (WS*W2)       -> h2 = HS*relu(pre2+b2): scale 1/WS
        h1 = mlp_layer(oh, 1, b1s, h1p, "h1", HS / WS)
        h2 = mlp_layer(h1, 2, b2s, h2p, "h2", 1.0 / WS)

        # ---- phase D: logits, exp, block-norms, gather, final reduce ----
        # psum3 = (HS*h2) @ (WS*W3) = HS*WS * logits
        # Software-pipelined so the PE never waits on ACT/DVE round trips:
        #  - stage(m): dense logits matmuls + (ACT exp, DVE scale+b3, DVE *oh)
        #  - tail(m):  dependent tiny PE matmuls, emitted one m behind
        lgs = 1.0 / (HS * WS)
        for c in range(n_ch):
            cs = slice(c * NCH, (c + 1) * NCH)
            ops = pso.tile([1, NCH], F32, name=f"ops_{s}_{c}", tag="ops")
            pn16 = psn.tile([16, NCH], F32, name=f"pn16_{s}_{c}", tag="pn16")
            exl, prl = {}, {}

            def stage(m):
                ps = psmm.tile([P, NCH], F32, name=f"lg_{s}_{c}_{m}", tag="ps")
                for kp in range(NKP):
                    nc.tensor.matmul(
                        ps[:],
                        wt[3, kp][:, :, m * P : (m + 1) * P],
                        h2[kp][:, :, cs],
                        start=(kp == 0),
                        stop=(kp == NKP - 1),
                        perf_mode=DR,
                    )
                # psum `ps` = HS*WS*logits (no b3); b3 enters via the Exp
                # bias (per-partition f32) and the DVE scale+add below.
                ex = exps.tile([P, NCH], BF16, name=f"ex_{s}_{c}_{m}", tag="ex")
                nc.scalar.activation(
                    ex[:],
                    ps[:],
                    mybir.ActivationFunctionType.Exp,
                    bias=b3f[:, m : m + 1],
                    scale=lgs,
                )
                tmp = prods.tile([P, NCH], BF16, name=f"tmp_{s}_{c}_{m}", tag="tmp")
                nc.vector.tensor_scalar(
                    tmp[:],
                    ps[:],
                    lgs,
                    b3f[:, m : m + 1],
                    mybir.AluOpType.mult,
                    mybir.AluOpType.add,
                )
                pr = prods.tile([P, NCH], BF16, name=f"pr_{s}_{c}_{m}", tag="pr")
                nc.vector.tensor_mul(pr[:], tmp[:], oh[m // 2][:, m % 2, cs])
                exl[m], prl[m] = ex, pr

            def tail(m):
                nc.tensor.matmul(
                    pn16[:],
                    blk16[:, m * 16 : (m + 1) * 16],
                    exl[m][:],
                    start=(m == 0),
                    stop=(m == NKT - 1),
                )
                nc.tensor.matmul(
                    ops[:], ones128[:], prl[m][:], start=(m == 0), stop=False
                )

            stage(0)
            stage(1)
            stage(2)
            stage(3)
            tail(0)
            emit_finish()  # previous chunk's reduction, behind 4 fresh kloops
            for m in range(4, NKT):
                stage(m)
                tail(m - 3)
            tail(NKT - 3)
            tail(NKT - 2)
            tail(NKT - 1)
            pending_finish[0] = (s, c, ops, pn16)
    emit_finish()

    ctx.close()


def build_nc(BC_=BC, NSC=2048, NCH=512):
    nc = bacc.Bacc("TRN2", target_bir_lowering=False, debug=False)
    t = {
        "ohdr": nc.dram_tensor("ohdr", [(BC_ // NSC) * (T // 2), 2, NSC], FP8, kind="ExternalInput"),
        "w1": nc.dram_tensor("w1", [T // 2, 2, H], FP8, kind="ExternalInput"),
        "w2": nc.dram_tensor("w2", [H // 2, 2, H], FP8, kind="ExternalInput"),
        "w3": nc.dram_tensor("w3", [H // 2, 2, T], FP8, kind="ExternalInput"),
        "b1r": nc.dram_tensor("b1r", [P, NKT], F32, kind="ExternalInput"),
        "b2r": nc.dram_tensor("b2r", [P, NKT], F32, kind="ExternalInput"),
        "b3f": nc.dram_tensor("b3f", [P, NKT], F32, kind="ExternalInput"),
        "blk16": nc.dram_tensor("blk16", [P, NKT * 16], BF16, kind="ExternalInput"),
        "out": nc.dram_tensor("out", [BC_ // NCH, NCH], F32, kind="ExternalOutput"),
    }
    with tile.TileContext(nc) as tc:
        _emit(tc, t, BC_, NSC, NCH)
    nc.compile()
    return nc


def _made_masks_np():
    in_deg = np.repeat(np.arange(D - 1), K)
    hid_deg = np.arange(H) % (D - 1)
    out_deg = np.repeat(np.arange(D), K)
    M1 = (hid_deg[None, :] >= in_deg[:, None]).astype(np.float32)
    M2 = (hid_deg[None, :] >= hid_deg[:, None]).astype(np.float32)
    M3 = (out_deg[None, :] > hid_deg[:, None]).astype(np.float32)
    return M1, M2, M3


def _pack_dr(wm):
    """[1024, C] f32 -> [512, 2, C] fp8 DoubleRow plane layout:
    out[128*kp + p, j, c] = WS * wm[128*(2*kp + j) + p, c]."""
    C = wm.shape[1]
    return np.ascontiguousarray(
        (WS * wm).reshape(NKP, 2, P, C).transpose(0, 2, 1, 3).reshape(NKP * P, 2, C)
    ).astype(FP8_NP)


def host_inputs(x, W1, b1, W2, b2, W3, b3, BC_=BC, n_cores=NCORES, NSC=2048):
    """Build the per-core in_maps (host-side prep: mask weights, expand x)."""
    x = np.asarray(x)
    M1, M2, M3 = _made_masks_np()
    w1m = np.zeros((H, H), dtype=np.float32)
    w1m[: T - K] = np.asarray(W1, np.float32) * M1
    w2m = np.asarray(W2, np.float32) * M2
    w3m = np.asarray(W3, np.float32) * M3
    b1r = (HS * np.asarray(b1, np.float32)).reshape(NKT, P).T.copy()
    b2r = (HS * np.asarray(b2, np.float32)).reshape(NKT, P).T.copy()
    b3c = np.asarray(b3, np.float32).reshape(NKT, P).T.copy()
    iota = (np.arange(T) % K).astype(np.int32)
    pp = np.arange(P) // K  # 0 for partitions 0..63, 1 for 64..127
    blk16 = np.zeros((P, NKT * 16), np.float32)
    for m in range(NKT):
        blk16[np.arange(P), 16 * m + 2 * m + pp] = 1.0
    blk16 = blk16.astype(BF16_NP)

    in_maps = []
    for c in range(n_cores):
        xs = x[c * BC_ : (c + 1) * BC_]  # [BC, D]
        xrep = np.repeat(xs.T.astype(np.int32), K, axis=0)  # [T, BC]
        ohf = (xrep == iota[:, None]).astype(FP8_NP)  # exact 0/1 one-hot
        # per-superchunk contiguous DoubleRow blocks:
        # rows (s*NKP+kp)*P + p, plane j, col n  <-  ohf[128*(2kp+j)+p, s*NSC+n]
        n_sc = BC_ // NSC
        ohdr = np.ascontiguousarray(
            ohf.reshape(NKP, 2, P, n_sc, NSC)
            .transpose(3, 0, 2, 1, 4)
            .reshape(n_sc * NKP * P, 2, NSC)
        )
        in_maps.append(
            {
                "ohdr": ohdr,
                "w1": _pack_dr(w1m),
                "w2": _pack_dr(w2m),
                "w3": _pack_dr(w3m),
                "b1r": b1r,
                "b2r": b2r,
                "b3f": b3c,
                "blk16": blk16,
            }
        )
    return in_maps


_NC_CACHE = {}


def kernel(x, W1, b1, W2, b2, W3, b3, **run_kwargs):
    if "nc" not in _NC_CACHE:
        _NC_CACHE["nc"] = build_nc()
    nc = _NC_CACHE["nc"]
    in_maps = host_inputs(x, W1, b1, W2, b2, W3, b3)
    res = run_bass_kernel_spmd(nc, in_maps, core_ids=list(range(NCORES)), **run_kwargs)
    out = np.concatenate([r["out"].reshape(-1) for r in res.results])
    if run_kwargs:
        kernel.last_results = res
    return out
